# revision 1
# baseline (speedup 1.0000x reference)
"""Trainium2 Bass kernel for grouped full attention with dynamic relative
position bias (8 heads, 400 tokens/group, dim 256, batch 128).

Strategy: pure data parallel over the 128 (batch*group) rows — 16 per core.
The tiny position-bias MLP runs on host (it only depends on the small weight
inputs); the device kernel computes, per batch group:
  qkT = (Wqk^T x^T), v = x Wv          (fp32r matmuls)
  S^T = K Q^T (scaled)                  (row-packed per head pair)
  E = exp(S^T) * exp(rpb^T)             (ACT exp + DVE/GPSIMD bf16 mul)
  U^T = V^T E, sums = 1^T E             (col-tiled 4 heads per matmul)
  out = (U / sums) @ Wp                 (fp32r)
"""

import math

import numpy as np
import ml_dtypes

import concourse.bass as bass
import concourse.mybir as mybir
import concourse.tile as tile
from concourse import bacc
from concourse.bass import ts
from concourse.bass_utils import run_bass_kernel_spmd

# ---- problem constants (hardcoded per contract) ----
T, V = 16, 25
N = T * V              # 400 tokens per group
DIM = 256
HEADS = 8
HEAD_DIM = 32
SCALE = HEAD_DIM ** -0.5
LN_EPS = 1e-5
B_ = 128
NCORES = 8
BPC = B_ // NCORES     # 16 batch groups per core
NPAD = 512             # token dim padded to 4 partition chunks
MC = 4                 # m chunks (128,128,128,16)

F32 = mybir.dt.float32
F32R = mybir.dt.float32r  # unused after bf16 switch
BF16 = mybir.dt.bfloat16

_CACHE = {}


def _pos_mlp_host(posproj_w, posproj_b, ln1_g, ln1_b, p1_w, p1_b,
                  ln2_g, ln2_b, p2_w, p2_b, ln3_g, ln3_b, p3_w, p3_b):
    """Replicates the reference dynamic position bias MLP in numpy fp32."""
    bh = np.arange(1 - T, T, dtype=np.float32)
    bw = np.arange(1 - V, V, dtype=np.float32)
    grid = np.stack(np.meshgrid(bh, bw, indexing="ij"))       # [2, 2T-1, 2V-1]
    biases = grid.reshape(2, -1).T.astype(np.float32)         # [(2T-1)(2V-1), 2]

    def layernorm(x, g, b):
        mu = x.mean(axis=-1, keepdims=True)
        var = x.var(axis=-1, keepdims=True)
        return (x - mu) / np.sqrt(var + LN_EPS) * g + b

    pos = biases @ posproj_w + posproj_b
    pos = np.maximum(layernorm(pos, ln1_g, ln1_b), 0.0) @ p1_w + p1_b
    pos = np.maximum(layernorm(pos, ln2_g, ln2_b), 0.0) @ p2_w + p2_b
    pos = np.maximum(layernorm(pos, ln3_g, ln3_b), 0.0) @ p3_w + p3_b
    return pos.astype(np.float32)                             # [(2T-1)(2V-1), HEADS]


def _rel_idx_host():
    coords = np.stack(np.meshgrid(np.arange(T), np.arange(V), indexing="ij"))
    cf = coords.reshape(2, -1)                                 # [2, N]
    rel = (cf[:, :, None] - cf[:, None, :]).transpose(1, 2, 0)  # [N, N, 2]
    rel[:, :, 0] += T - 1
    rel[:, :, 1] += V - 1
    rel[:, :, 0] *= 2 * V - 1
    return rel.sum(-1).astype(np.int32)                        # [N, N]


def _emit(ctx, tc, d, vb_nonzero, pb_nonzero, bpc):
    nc = tc.nc

    const = ctx.enter_context(tc.tile_pool(name="const", bufs=1))
    xt_pool = ctx.enter_context(tc.tile_pool(name="xt", bufs=3))
    qk_pool = ctx.enter_context(tc.tile_pool(name="qk", bufs=10))
    v_pool = ctx.enter_context(tc.tile_pool(name="v", bufs=3))
    e_pool = ctx.enter_context(tc.tile_pool(name="e", bufs=34))
    rs_pool = ctx.enter_context(tc.tile_pool(name="rs", bufs=3))
    un_pool = ctx.enter_context(tc.tile_pool(name="un", bufs=3))
    # 8 psum banks total: ps_gen 2x[128,2,512] for scores/qk/v/proj units,
    # ps_av 2x[128,2,512] for the per-quad U^T+sums accumulators.
    ps_gen = ctx.enter_context(tc.tile_pool(name="ps_gen", bufs=2, space="PSUM"))
    ps_av = ctx.enter_context(tc.tile_pool(name="ps_av", bufs=2, space="PSUM"))

    # ---- resident constants ----
    w_qk = const.tile([128, 2, 512], BF16)
    nc.sync.dma_start(w_qk[:], d["w_qk"][:])
    w_v = const.tile([128, 2, 256], BF16)
    nc.sync.dma_start(w_v[:], d["w_v"][:])
    w_p = const.tile([128, 2, 256], BF16)
    nc.sync.dma_start(w_p[:], d["w_p"][:])
    bqk = const.tile([128, 4], F32)
    nc.sync.dma_start(bqk[:], d["bqk"][:])
    rb = const.tile([128, HEADS, MC, N], BF16)
    nc.sync.dma_start(rb[:], d["rbias"][:])
    rb3 = const.tile([128, 2, N], BF16)
    nc.sync.dma_start(rb3[:], d["rbias3"][:])
    ones = const.tile([128, 32], BF16)
    nc.vector.memset(ones[:], 1.0)
    if vb_nonzero:
        vb = const.tile([128, 256], F32)
        nc.sync.dma_start(
            vb[:],
            bass.AP(tensor=d["bv"].tensor, offset=d["bv"].offset,
                    ap=[[0, 128]] + d["bv"].ap),
        )
    if pb_nonzero:
        pb = const.tile([128, 256], F32)
        nc.sync.dma_start(
            pb[:],
            bass.AP(tensor=d["bp"].tensor, offset=d["bp"].offset,
                    ap=[[0, 128]] + d["bp"].ap),
        )
    out_pool = ctx.enter_context(tc.tile_pool(name="outp", bufs=3))

    def emit_qkv(b):
        xt = xt_pool.tile([128, 2, N], BF16)
        nc.sync.dma_start(xt[:], d["xt"][b])
        qkT = []
        for t in range(4):
            ps = ps_gen.tile([128, 2, 512], F32, tag="u")
            for cc in range(2):
                nc.tensor.matmul(
                    ps[:, 0, 0:N], w_qk[:, cc, ts(t, 128)], xt[:, cc, :],
                    start=(cc == 0), stop=(cc == 1),
                )
            qt = qk_pool.tile([128, N], BF16, tag="qkT")
            nc.scalar.activation(
                out=qt[:], in_=ps[:, 0, 0:N],
                func=mybir.ActivationFunctionType.Identity,
                bias=bqk[:, t:t + 1],
            )
            qkT.append(qt)
        v = v_pool.tile([128, MC, 256], BF16)
        for nt in range(4):
            m = 128 if nt < 3 else N - 3 * 128
            ps = ps_gen.tile([128, 2, 512], F32, tag="u")
            for cc in range(2):
                nc.tensor.matmul(
                    ps[0:m, 0, 0:256],
                    xt[:, cc, nt * 128:nt * 128 + m],
                    w_v[:, cc, :],
                    start=(cc == 0), stop=(cc == 1),
                )
            if vb_nonzero:
                nc.vector.tensor_tensor(
                    out=v[0:m, nt, :], in0=ps[0:m, 0, 0:256], in1=vb[0:m, :],
                    op=mybir.AluOpType.add)
            else:
                nc.scalar.copy(out=v[0:m, nt, :], in_=ps[0:m, 0, 0:256])
        v3 = v_pool.tile([128, 256], BF16, tag="v3")
        for j in range(4):
            nc.sync.dma_start(v3[32 * j:32 * j + 16, :], v[112:128, 2, :])
            nc.sync.dma_start(v3[32 * j + 16:32 * j + 32, :], v[0:16, 3, :])
        return qkT, v, v3

    def emit_main(b, qkT, Eqprev, vprev, v3prev):
        """Scores+exp+mul for batch b interleaved with AV/sums for b-1.
        mc 0..2 run per (quad, head-pair); the 16-row token tails of all
        8 heads merge into one cross-quad unit (row+col tile_position)."""
        Eq = {}
        unT = recS = None
        if Eqprev is not None:
            unT = un_pool.tile([128, 2, N], BF16, tag="unT")
            recS = rs_pool.tile([128, 2, N], F32, tag="recS")
        avs = []
        for q in range(2):
            if Eqprev is not None:
                av = ps_av.tile([128, 2, 512], F32, tag="av")
                avs.append(av)
            for mc in range(3):
                m = 128 if mc < 2 else 112
                k = m
                for pair in range(2):
                    ps = ps_gen.tile([128, 2, 512], F32, tag="u")
                    for i in range(2):
                        h = 4 * q + 2 * pair + i
                        rbase = 32 * (h % 4)
                        nc.tensor.matmul(
                            ps[0:m, i, 0:N],
                            qkT[2 + h // 4][rbase:rbase + 32,
                                            mc * 128:mc * 128 + m],
                            qkT[h // 4][rbase:rbase + 32, :],
                            start=True, stop=True,
                            tile_position=(rbase, 0),
                        )
                    h0 = 4 * q + 2 * pair
                    eu = e_pool.tile([128, 2, N], BF16, tag="eu")
                    nc.scalar.activation(
                        out=eu[0:m, :, :],
                        in_=ps[0:m, :, 0:N],
                        func=mybir.ActivationFunctionType.Exp,
                    )
                    eng = nc.gpsimd if (2 * mc + pair) % 3 == 1 else nc.vector
                    eng.tensor_tensor(
                        out=eu[0:m, :, :],
                        in0=eu[0:m, :, :],
                        in1=rb[0:m, h0:h0 + 2, mc, :],
                        op=mybir.AluOpType.mult,
                    )
                    Eq[(q, mc, pair)] = eu
                    if Eqprev is not None:
                        bank = pair
                        for h4 in range(4):
                            h = 4 * q + h4
                            lhs = (vprev[0:k, mc, 32 * h:32 * h + 32]
                                   if bank == 0 else ones[0:k, :])
                            nc.tensor.matmul(
                                av[32 * h4:32 * h4 + 32, bank, 0:N],
                                lhs,
                                Eqprev[(q, mc, h4 // 2)][0:k, h4 % 2, :],
                                start=(mc == 0), stop=False,
                                tile_position=(0, 32 * h4),
                                skip_group_check=True,
                            )
        # merged token-tail unit: all 8 heads' m=384:400 scores into one
        # 2-bank slot at (row, col) = (32j, 32j), one exp + one mul
        ps3 = ps_gen.tile([128, 2, 512], F32, tag="u", name="ps3")
        for q in range(2):
            for j in range(4):
                h = 4 * q + j
                nc.tensor.matmul(
                    ps3[32 * j:32 * j + 32, q, 0:N],
                    qkT[2 + h // 4][32 * j:32 * j + 32, 368:400],
                    qkT[h // 4][32 * j:32 * j + 32, :],
                    start=True, stop=True,
                    tile_position=(32 * j, 32 * j),
                    skip_group_check=True,
                )
        e3 = e_pool.tile([128, 2, N], BF16, tag="eu")
        nc.scalar.activation(
            out=e3[:, :, :], in_=ps3[:, :, 0:N],
            func=mybir.ActivationFunctionType.Exp,
        )
        nc.gpsimd.tensor_tensor(
            out=e3[:, :, :], in0=e3[:, :, :], in1=rb3[:, :, :],
            op=mybir.AluOpType.mult,
        )
        Eq["tail"] = e3
        # tail AV/sums packs for b-1 (close the accumulation groups)
        if Eqprev is not None:
            e3p = Eqprev["tail"]
            for q in range(2):
                av = avs[q]
                for bank in range(2):
                    for j in range(4):
                        h = 4 * q + j
                        lhs = (v3prev[32 * j:32 * j + 32, 32 * h:32 * h + 32]
                               if bank == 0 else ones[32 * j:32 * j + 32, :])
                        nc.tensor.matmul(
                            av[32 * j:32 * j + 32, bank, 0:N],
                            lhs,
                            e3p[32 * j:32 * j + 32, q, :],
                            start=False, stop=True,
                            tile_position=(32 * j, 32 * j),
                            skip_group_check=True,
                        )
                nc.vector.reciprocal_approx_fast(
                    out=recS[:, q, :], in_=av[:, 1, 0:N])
                nc.vector.tensor_tensor(
                    out=unT[:, q, :], in0=av[:, 0, 0:N], in1=recS[:, q, :],
                    op=mybir.AluOpType.mult,
                )
        return Eq, unT

    def emit_main_drain(prev):
        Eqprev, vprev, v3prev = prev
        unT = un_pool.tile([128, 2, N], BF16, tag="unT")
        recS = rs_pool.tile([128, 2, N], F32, tag="recS")
        e3p = Eqprev["tail"]
        for q in range(2):
            av = ps_av.tile([128, 2, 512], F32, tag="av")
            for mc in range(3):
                k = 128 if mc < 2 else 112
                for bank in range(2):
                    for h4 in range(4):
                        h = 4 * q + h4
                        lhs = (vprev[0:k, mc, 32 * h:32 * h + 32]
                               if bank == 0 else ones[0:k, :])
                        nc.tensor.matmul(
                            av[32 * h4:32 * h4 + 32, bank, 0:N],
                            lhs,
                            Eqprev[(q, mc, h4 // 2)][0:k, h4 % 2, :],
                            start=(mc == 0), stop=False,
                            tile_position=(0, 32 * h4),
                            skip_group_check=True,
                        )
            for bank in range(2):
                for j in range(4):
                    h = 4 * q + j
                    lhs = (v3prev[32 * j:32 * j + 32, 32 * h:32 * h + 32]
                           if bank == 0 else ones[32 * j:32 * j + 32, :])
                    nc.tensor.matmul(
                        av[32 * j:32 * j + 32, bank, 0:N],
                        lhs,
                        e3p[32 * j:32 * j + 32, q, :],
                        start=False, stop=True,
                        tile_position=(32 * j, 32 * j),
                        skip_group_check=True,
                    )
            nc.vector.reciprocal_approx_fast(
                out=recS[:, q, :], in_=av[:, 1, 0:N])
            nc.vector.tensor_tensor(
                out=unT[:, q, :], in0=av[:, 0, 0:N], in1=recS[:, q, :],
                op=mybir.AluOpType.mult,
            )
        return None, unT

    def emit_proj(b, unT):
        for nt in range(4):
            m = 128 if nt < 3 else N - 3 * 128
            ps = ps_av.tile([128, 2, 512], F32, tag="av")
            for cc in range(2):
                nc.tensor.matmul(
                    ps[0:m, 0, 0:256],
                    unT[:, cc, nt * 128:nt * 128 + m],
                    w_p[:, cc, :],
                    start=(cc == 0), stop=(cc == 1),
                )
            o = out_pool.tile([128, 256], F32)
            if pb_nonzero:
                nc.vector.tensor_tensor(
                    out=o[0:m, :], in0=ps[0:m, 0, 0:256], in1=pb[0:m, :],
                    op=mybir.AluOpType.add)
            else:
                nc.vector.tensor_copy(out=o[0:m, :], in_=ps[0:m, 0, 0:256])
            nc.sync.dma_start(d["out"][b, nt * 128:nt * 128 + m], o[0:m, :])

    # 3-stage software pipeline with PE interleaving: scores(b) and
    # AV(b-1) share one emission pass; proj(b-2) slots in after qkv(b).
    prev = None          # (Eq, v, v3) of b-1
    unT_hold = {}
    for b in range(bpc):
        qkT, v, v3 = emit_qkv(b)
        if b >= 2:
            emit_proj(b - 2, unT_hold.pop(b - 2))
        Eq, unT = emit_main(b, qkT,
                            prev[0] if prev else None,
                            prev[1] if prev else None,
                            prev[2] if prev else None)
        if unT is not None:
            unT_hold[b - 1] = unT
        prev = (Eq, v, v3)
    _, unT = emit_main_drain(prev)
    unT_hold[bpc - 1] = unT
    for b in (bpc - 2, bpc - 1):
        if b in unT_hold:
            emit_proj(b, unT_hold.pop(b))


def _build(vb_nonzero, pb_nonzero, bpc=BPC):
    nc = bacc.Bacc("TRN2", target_bir_lowering=False, debug=False,
                   num_devices=NCORES)
    d = {}
    d["xt"] = nc.dram_tensor("xt", [bpc, 128, 2, N], BF16,
                             kind="ExternalInput").ap()
    d["w_qk"] = nc.dram_tensor("w_qk", [128, 2, 512], BF16,
                               kind="ExternalInput").ap()
    d["w_v"] = nc.dram_tensor("w_v", [128, 2, 256], BF16,
                              kind="ExternalInput").ap()
    d["w_p"] = nc.dram_tensor("w_p", [128, 2, 256], BF16,
                              kind="ExternalInput").ap()
    d["bqk"] = nc.dram_tensor("bqk", [128, 4], F32,
                              kind="ExternalInput").ap()
    d["rbias"] = nc.dram_tensor("rbias", [128, HEADS, MC, N], BF16,
                                kind="ExternalInput").ap()
    d["rbias3"] = nc.dram_tensor("rbias3", [128, 2, N], BF16,
                                 kind="ExternalInput").ap()
    if vb_nonzero:
        d["bv"] = nc.dram_tensor("bv", [256], F32, kind="ExternalInput").ap()
    if pb_nonzero:
        d["bp"] = nc.dram_tensor("bp", [256], F32, kind="ExternalInput").ap()
    d["out"] = nc.dram_tensor("out", [bpc, N, DIM], F32,
                              kind="ExternalOutput").ap()

    from contextlib import ExitStack

    with tile.TileContext(nc) as tc:
        with ExitStack() as ctx:
            _emit(ctx, tc, d, vb_nonzero, pb_nonzero, bpc)
    nc.compile()
    return nc, d


def _prep_host(inputs):
    x = np.ascontiguousarray(np.asarray(inputs["x"], dtype=np.float32))
    qkv_w = np.asarray(inputs["qkv_w"], dtype=np.float32)
    qkv_b = np.asarray(inputs["qkv_b"], dtype=np.float32)
    proj_w = np.asarray(inputs["proj_w"], dtype=np.float32)
    proj_b = np.asarray(inputs["proj_b"], dtype=np.float32)

    pos = _pos_mlp_host(
        *[np.asarray(inputs[k], dtype=np.float32) for k in (
            "posproj_w", "posproj_b", "ln1_g", "ln1_b", "p1_w", "p1_b",
            "ln2_g", "ln2_b", "p2_w", "p2_b", "ln3_g", "ln3_b",
            "p3_w", "p3_b")])
    rel = _rel_idx_host()
    rpb = pos[rel.reshape(-1)].reshape(N, N, HEADS)       # [n, m, h]
    rbiasT = np.exp(rpb.transpose(2, 1, 0))               # [h, m, n]
    rpad = np.ones((HEADS, NPAD, N), np.float32)
    rpad[:, :N, :] = rbiasT
    rbias_dev = np.ascontiguousarray(
        rpad.reshape(HEADS, MC, 128, N).transpose(2, 0, 1, 3)
    ).astype(ml_dtypes.bfloat16)                          # [128, h, mc, n]
    # merged token-tail table: rows 32j+p (p<16) hold head 4q+j, m=384+p
    rb3 = np.ones((128, 2, N), np.float32)
    for qq in range(2):
        for j in range(4):
            rb3[32 * j:32 * j + 32, qq, :] = rbiasT[4 * qq + j, 368:400, :]
    rbias3_dev = np.ascontiguousarray(rb3).astype(ml_dtypes.bfloat16)

    w_qk = qkv_w[:, :512].copy()
    w_qk[:, :256] *= SCALE
    w_qk_dev = np.ascontiguousarray(
        w_qk.reshape(2, 128, 512).transpose(1, 0, 2)).astype(ml_dtypes.bfloat16)
    w_v_dev = np.ascontiguousarray(
        qkv_w[:, 512:].reshape(2, 128, 256).transpose(1, 0, 2)).astype(
        ml_dtypes.bfloat16)
    w_p_dev = np.ascontiguousarray(
        proj_w.reshape(2, 128, 256).transpose(1, 0, 2)).astype(
        ml_dtypes.bfloat16)
    b_qk = qkv_b[:512].copy()
    b_qk[:256] *= SCALE
    bqk_dev = np.ascontiguousarray(b_qk.reshape(4, 128).T)

    b_v = qkv_b[512:]
    vb_nonzero = bool(np.any(b_v != 0))
    pb_nonzero = bool(np.any(proj_b != 0))

    # x^T per core: [BPC, 128 (c within chunk), 2 (chunk), 400]
    xt_all = np.ascontiguousarray(
        x.transpose(0, 2, 1).reshape(B_, 2, 128, N).transpose(0, 2, 1, 3)
    ).astype(ml_dtypes.bfloat16)

    common = {
        "w_qk": w_qk_dev, "w_v": w_v_dev, "w_p": w_p_dev,
        "bqk": bqk_dev, "rbias": rbias_dev, "rbias3": rbias3_dev,
    }
    if vb_nonzero:
        common["bv"] = np.ascontiguousarray(b_v)
    if pb_nonzero:
        common["bp"] = np.ascontiguousarray(proj_b)
    in_maps = []
    for c in range(NCORES):
        m = dict(common)
        m["xt"] = np.ascontiguousarray(xt_all[c * BPC:(c + 1) * BPC])
        in_maps.append(m)
    return in_maps, vb_nonzero, pb_nonzero


def kernel(**inputs) -> np.ndarray:
    in_maps, vb_nonzero, pb_nonzero = _prep_host(inputs)
    key = (vb_nonzero, pb_nonzero)
    if key not in _CACHE:
        _CACHE[key] = _build(vb_nonzero, pb_nonzero)
    nc, _ = _CACHE[key]
    res = run_bass_kernel_spmd(nc, in_maps, core_ids=list(range(NCORES)))
    out = np.concatenate([res.results[c]["out"] for c in range(NCORES)], axis=0)
    return out.astype(np.float32)


def run_traced(**inputs):
    """Like kernel() but with NTFF tracing; returns (out, BassKernelResults)."""
    in_maps, vb_nonzero, pb_nonzero = _prep_host(inputs)
    key = (vb_nonzero, pb_nonzero)
    if key not in _CACHE:
        _CACHE[key] = _build(vb_nonzero, pb_nonzero)
    nc, _ = _CACHE[key]
    res = run_bass_kernel_spmd(nc, in_maps, core_ids=list(range(NCORES)),
                               trace=True)
    out = np.concatenate([res.results[c]["out"] for c in range(NCORES)], axis=0)
    return out.astype(np.float32), res

